# revision 23
# baseline (speedup 1.0000x reference)
"""ClassicalSelfAttention Trainium2 kernel, 8-core SPMD.

Math (reference):
    q = (x @ W_rot.T).reshape(B, D, 3)        # B=32, D=2048
    k = (x @ W_ent.T).reshape(B, D, 3)
    S[b,d,e] = sum_c q[b,d,c] k[b,e,c] / sqrt(D)
    out[b,d] = sum_e softmax_e(S)[b,d,e] * x[b,e]

Sharding: core m owns d in [256m, 256(m+1)) == rows [768m, 768(m+1)) of both
weight matrices (6MB/core HBM instead of 96MB replicated).  Each core
computes its q-shard and k-shard, AllGathers k (49KB/rank bf16), then streams
e-tiles flash-style: scores^T matmul (batch-packed block-diagonal, K=12)
-> exp on ScalarE -> num/den reduction matmul against [x, 1] columns ->
divide -> its 256 output columns.  Softmax skips the max-subtraction:
|S| < ~2 here (q,k are unit-scale and S carries 1/sqrt(D)), so exp is safe.

All matmul moving operands are bf16 (1 cycle/row on the PE and no fp32
power-throttle); PSUM accumulation is fp32.  A tiny pre-barrier collective
at kernel start absorbs cross-core launch skew under the weight DMAs so
the k AllGather doesn't eat it.
"""

import numpy as np

import concourse.bass as bass
import concourse.mybir as mybir
import concourse.tile as tile
from concourse import bacc
from concourse.bass_utils import run_bass_kernel_spmd

B, D = 32, 2048
NC = 8
DSH = D // NC  # 256 d-values per core
JSH = 3 * DSH  # 768 weight rows per core
KT = D // 128  # 16 contraction tiles for projections
CH = 8  # batch chunks in main loop
CB = B // CH  # 4 batches per chunk
KROWS = 3 * CB  # 12 stacked contraction rows per chunk
CW = CB * DSH  # 1024 score columns per chunk
ET = D // 128  # 16 e-tiles
F32 = mybir.dt.float32
F32R = mybir.dt.float32r
BF16 = mybir.dt.bfloat16

_CACHE: dict = {}


def _build(sim=False, reps=1):
    # sim=True: single-core collective-free variant for TimelineSim cost runs
    nc = bacc.Bacc("TRN2", num_devices=(1 if sim else NC))

    # Host-prepped layouts (partition-major, dense DMA):
    #   xT   [128, KT*B]   : col = kt*32 + b,   part = f % 128, f = 128*kt + p
    #   xw   [128, ET*64]  : col = 64*et + 8*j + cc; cc<4 -> x[4j+cc, e], else 1.0
    #   wrot [128, KT*JSH] : col = kt*768 + j_local (W_rot shard, pre-scaled, .T)
    #   went [128, KT*JSH] : same for W_ent (unscaled)
    xT = nc.dram_tensor("xT", [128, KT * B], BF16, kind="ExternalInput")
    xw = nc.dram_tensor("xw", [128, ET * 64], BF16, kind="ExternalInput")
    wrot = nc.dram_tensor("wrot", [128, KT * JSH], BF16, kind="ExternalInput")
    went = nc.dram_tensor("went", [128, KT * JSH], BF16, kind="ExternalInput")
    out = nc.dram_tensor("out", [B, DSH], F32, kind="ExternalOutput")

    # DRAM scratch.  Weight shards are host-permuted to c-major row order
    # (j' = 256c + d), so y_ent rows are already [c, e_l] grouped and the
    # k-shard export is a dense copy: ag_in row (3b+c) = y_ent[b, 256c:...].
    ag_in = nc.dram_tensor("ag_in", [3 * B, DSH], BF16)  # rows 3b+c
    ag_out = nc.dram_tensor("ag_out", [NC * 3 * B, DSH], BF16, addr_space="Shared")
    # observable sink for the HAM-bridge filler accumulation (see below)
    flush = nc.dram_tensor("flush", [1, 16], F32)

    ExpF = mybir.ActivationFunctionType.Exp

    with tile.TileContext(nc) as tc:
        with (
            tc.tile_pool(name="const", bufs=1) as const,
            tc.tile_pool(name="wp", bufs=4) as wp,
            tc.tile_pool(name="ysb", bufs=1) as ysb,
        ):
            xT_sb = const.tile([128, KT * B], BF16, tag="xT_sb")
            nc.sync.dma_start(out=xT_sb, in_=xT[:, :])
            xw_sb = const.tile([128, ET * 64], BF16, tag="xw_sb")
            nc.scalar.dma_start(out=xw_sb, in_=xw[:, :])
            # q stack: [96, 8192] block-diagonal (rows 3b+c; col g*1024+
            # (b%4)*256+d nonzero only when b//4 == g).  K=96 keeps the PE
            # cell-occupancy high so the HAM clock gate stays at 8/8.
            q_sb = const.tile([3 * B, CH * CW], BF16, tag="q_sb")
            nc.vector.memset(q_sb[:, :], 0.0)
            # k stack: [96, 2048], rows 3b+c over all batches
            k_sb = const.tile([3 * B, D], BF16, tag="k_sb")

            # ---- projections: ent first (unblocks AllGather), then rot ----
            # weight chunks round-robin on all 3 DMA-capable queues (~70GB/s
            # per queue) so the ent projection -> AllGather trigger is early
            y_sb = {}
            CKT = 2  # kts per weight chunk
            with tc.tile_pool(name="yps", bufs=1, space="PSUM") as yps:
                for wname, wdram in (("ent", went), ("rot", wrot)):
                    y_ps = yps.tile([B, JSH], F32, tag=f"y_{wname}")
                    for kg in range(KT // CKT):
                        w_t = wp.tile([128, CKT * JSH], BF16, tag="w_t")
                        deng = [nc.sync, nc.scalar, nc.gpsimd][kg % 3]
                        deng.dma_start(
                            out=w_t,
                            in_=wdram[:, CKT * kg * JSH : CKT * (kg + 1) * JSH],
                        )
                        for kk in range(CKT):
                            kt = CKT * kg + kk
                            lhs = xT_sb[:, kt * B : (kt + 1) * B]
                            nc.tensor.matmul(
                                y_ps[:, 0:512],
                                lhs,
                                w_t[:, kk * JSH : kk * JSH + 512],
                                start=(kt == 0),
                                stop=(kt == KT - 1),
                            )
                            nc.tensor.matmul(
                                y_ps[:, 512:JSH],
                                lhs,
                                w_t[:, kk * JSH + 512 : (kk + 1) * JSH],
                                start=(kt == 0),
                                stop=(kt == KT - 1),
                            )
                    y_sb[wname] = ysb.tile(
                        [B, JSH], BF16, tag=f"ysb_{wname}", name=f"ysb_{wname}"
                    )
                    nc.vector.tensor_copy(out=y_sb[wname], in_=y_ps)

                    if wname == "ent":
                        # k-shard export: dense copy thanks to c-major W rows
                        nc.sync.dma_start(out=ag_in[:, :], in_=y_sb["ent"])
                        if sim:
                            for r in range(NC):
                                nc.sync.dma_start(
                                    out=ag_out[3 * B * r : 3 * B * (r + 1), :],
                                    in_=ag_in[:, :],
                                )
                        else:
                            nc.gpsimd.collective_compute(
                                "AllGather",
                                mybir.AluOpType.bypass,
                                replica_groups=[list(range(NC))],
                                ins=[ag_in[:, :].opt()],
                                outs=[ag_out[:, :].opt()],
                            )

            # block-diagonal scatter, per-(j,b2), round-robin queues:
            #   q_sb[12j+3b2+c, 1024j + 256b2 + d] = y_rot[4j+b2, 256c + d]
            for j in range(CH):
                for b2 in range(CB):
                    src = y_sb["rot"][CB * j + b2 : CB * j + b2 + 1, :].rearrange(
                        "p (c d) -> p c d", c=3
                    )
                    row = 12 * j + 3 * b2
                    [nc.gpsimd, nc.sync, nc.scalar][(CB * j + b2) % 3].dma_start(
                        out=q_sb[
                            row : row + 3,
                            j * CW + b2 * DSH : j * CW + (b2 + 1) * DSH,
                        ],
                        in_=src,
                    )

            # k-stack build, per-rank so et=0 (rank 0) unblocks first:
            #   k_sb[row, 256r + e_l] = ag_out[96r + row, e_l]
            for r in range(NC):
                src = bass.AP(
                    tensor=ag_out.ap().tensor,
                    offset=3 * B * DSH * r,
                    ap=[[DSH, 3 * B], [1, DSH]],
                )
                [nc.sync, nc.scalar, nc.gpsimd][r % 3].dma_start(
                    out=k_sb[:, DSH * r : DSH * (r + 1)],
                    in_=src,
                )

            # HAM bridge: the rot projection earns the 8/8 PE clock grant
            # (big K=128 matmuls); the AllGather wait would idle the PE long
            # enough to lose it.  Keep the PE streaming high-occupancy filler
            # matmuls until k_sb is ready.  All fillers accumulate into one
            # PSUM group feeding an observable DRAM write, so none are DCE'd.
            # (If they drain early, the main loop's own K=96/M=64 matmuls
            # re-earn the grant within ~2 HAM windows — bounded downside.)
            if not sim:
                NFILL = 100
                with (
                    tc.tile_pool(name="fps", bufs=1, space="PSUM") as fps,
                    tc.tile_pool(name="fsb", bufs=1) as fsb,
                ):
                    f_ps = fps.tile([B, 512], F32, tag="f_ps")
                    for f in range(NFILL):
                        nc.tensor.matmul(
                            f_ps[:, :],
                            xT_sb[:, 0:B],
                            xT_sb[:, 0:512],
                            start=(f == 0),
                            stop=(f == NFILL - 1),
                        )
                    f_sb = fsb.tile([1, 16], F32, tag="f_sb")
                    nc.vector.tensor_copy(out=f_sb, in_=f_ps[0:1, 0:16])
                    nc.sync.dma_start(out=flush[:, :], in_=f_sb)

            # ---- main loop: flash-style streaming over e-tiles ----
            with (
                tc.tile_pool(name="sps", bufs=2, space="PSUM") as sps,
                tc.tile_pool(name="aps", bufs=2, space="PSUM") as aps,
                tc.tile_pool(name="ep", bufs=2) as ep,
                tc.tile_pool(name="ev", bufs=3) as ev,
            ):
                # num rows 0-31, den rows 32-63
                nd_all = const.tile([2 * B, DSH], F32, tag="nd_all", name="nd_all")

                def phase_c(burst=False):
                  for j in range(CH):
                    acc_ps = aps.tile([64, CW], F32, tag="acc", name="acc_ps")
                    pend = [None] * ET  # deferred accum inputs
                    for et in range(ET):
                        s_ps = sps.tile([128, CW], F32, tag="s", name="s_ps")
                        k_sl = k_sb[:, et * 128 : (et + 1) * 128]
                        for h in range(2):
                            nc.tensor.matmul(
                                s_ps[:, h * 512 : (h + 1) * 512],
                                k_sl,
                                q_sb[
                                    :, j * CW + h * 512 : j * CW + (h + 1) * 512
                                ],
                                start=True,
                                stop=True,
                            )
                        e_sb = ev.tile([128, CW], BF16, tag="e_sb", name="e_sb")
                        nc.scalar.activation(out=e_sb, in_=s_ps, func=ExpF)
                        pend[et] = e_sb
                        # deferred: accumulate the PREVIOUS e-tile, so the PE
                        # never waits on the exp that was just issued.  The
                        # stationary is the full 64-col xw slice (M=64 keeps
                        # PE occupancy high); rows 8j..8j+8 carry group j's
                        # num/den, other rows are ignored garbage.
                        for ep_i in ([et - 1] if et > 0 else []):
                            xw_sl = xw_sb[:, ep_i * 64 : (ep_i + 1) * 64]
                            for h in range(2):
                                nc.tensor.matmul(
                                    acc_ps[:, h * 512 : (h + 1) * 512],
                                    xw_sl,
                                    pend[ep_i][:, h * 512 : (h + 1) * 512],
                                    start=(ep_i == 0),
                                    stop=False,
                                )
                    xw_sl = xw_sb[:, (ET - 1) * 64 : ET * 64]
                    for h in range(2):
                        nc.tensor.matmul(
                            acc_ps[:, h * 512 : (h + 1) * 512],
                            xw_sl,
                            pend[ET - 1][:, h * 512 : (h + 1) * 512],
                            start=False,
                            stop=True,
                        )
                    # epilogue: copy PSUM->SBUF once, then trickle the
                    # num+den diagonal blocks into nd_all via small DMAs
                    acc_sb = ep.tile([64, CW], F32, tag="acc_sb", name="acc_sb")
                    nc.vector.tensor_copy(out=acc_sb, in_=acc_ps)
                    for b2 in range(CB):
                        row = CB * j + b2
                        eng = [nc.sync, nc.gpsimd][(CB * j + b2) % 2]
                        eng.dma_start(
                            out=nd_all[row : row + 1, :],
                            in_=acc_sb[
                                8 * j + b2 : 8 * j + b2 + 1,
                                b2 * DSH : (b2 + 1) * DSH,
                            ],
                        )
                        eng2 = [nc.gpsimd, nc.sync][(CB * j + b2) % 2]
                        eng2.dma_start(
                            out=nd_all[B + row : B + row + 1, :],
                            in_=acc_sb[
                                8 * j + 4 + b2 : 8 * j + 4 + b2 + 1,
                                b2 * DSH : (b2 + 1) * DSH,
                            ],
                        )

                  # final: one reciprocal + multiply + output DMA
                  rec_sb = ep.tile([B, DSH], F32, tag="rec_sb", name="rec_sb")
                  nc.vector.reciprocal(out=rec_sb, in_=nd_all[B : 2 * B, :])
                  o_sb = ep.tile([B, DSH], F32, tag="o_sb", name="o_sb")
                  nc.vector.tensor_mul(o_sb, nd_all[0:B, :], rec_sb)
                  nc.sync.dma_start(out=out[:, :], in_=o_sb)

                if reps <= 32:
                    for r in range(reps):
                        phase_c(burst=(r == 0))
                else:
                    phase_c(burst=True)
                    with tc.For_i(0, reps - 1, 1):
                        phase_c()

    nc.compile()
    return nc


def _prep_inputs(x, W_rot, W_ent):
    """Host-side shard + layout prep (pure reshapes/transposes + one scale)."""
    scale = np.float32(1.0 / np.sqrt(np.float32(D)))
    xT = np.ascontiguousarray(x.T)  # [2048, 32]
    import ml_dtypes

    xT_prep = np.ascontiguousarray(
        xT.reshape(KT, 128, B).transpose(1, 0, 2).reshape(128, KT * B)
    ).astype(ml_dtypes.bfloat16)
    # xw[p, 64*et + 8*j + cc]
    xe = xT.reshape(ET, 128, B)  # [et, p, b]
    A = np.ones((ET, 128, CH, 2 * CB), dtype=np.float32)
    A[:, :, :, 0:CB] = xe.reshape(ET, 128, CH, CB)
    xw_prep = np.ascontiguousarray(
        A.transpose(1, 0, 2, 3).reshape(128, ET * 64)
    ).astype(ml_dtypes.bfloat16)

    def wprep(W, m, do_scale):
        sh = W[JSH * m : JSH * (m + 1), :]
        if do_scale:
            sh = sh * scale
        # c-major row permutation: new row j' = 256c + d holds old row 3d + c
        sh = sh.reshape(DSH, 3, D).transpose(1, 0, 2).reshape(JSH, D)
        return np.ascontiguousarray(
            sh.T.reshape(KT, 128, JSH).transpose(1, 0, 2).reshape(128, KT * JSH)
        ).astype(ml_dtypes.bfloat16)

    in_maps = []
    for m in range(NC):
        in_maps.append(
            {
                "xT": xT_prep,
                "xw": xw_prep,
                "wrot": wprep(W_rot, m, True),
                "went": wprep(W_ent, m, False),
            }
        )
    return in_maps


def kernel(x, W_rot, W_ent):
    x = np.asarray(x, dtype=np.float32)
    W_rot = np.asarray(W_rot, dtype=np.float32)
    W_ent = np.asarray(W_ent, dtype=np.float32)
    if "nc" not in _CACHE:
        _CACHE["nc"] = _build()
    nc = _CACHE["nc"]
    in_maps = _prep_inputs(x, W_rot, W_ent)
    res = run_bass_kernel_spmd(nc, in_maps, core_ids=list(range(NC)))
    full = np.empty((B, D), dtype=np.float32)
    for m in range(NC):
        full[:, DSH * m : DSH * (m + 1)] = res.results[m]["out"]
    return full
